# revision 37
# baseline (speedup 1.0000x reference)
"""Trainium2 Bass kernel for nn_MultiHeadAttention_80418967650946.

Reference computation (per batch b):
  qp/kp/vp = 1x1-conv projections of q/k/v   [64, N]
  funky head view: qh[h,n,d] = qp.reshape(4, 16*N)[d, 16n+h]  (same for kh, vh)
  scores = qh @ kh * 0.25^0.5 + bias ; attn = softmax(scores)
  x[4h+d, n] = (attn @ vh)[h, n, d] ; y = LeakyReLU(BN(Wo @ x + bo), 0.2)

Sharding: 8 cores = 4 batches x 2 query-halves (n in [0,512) or [512,1024)).
Each core computes its query-half for ALL 16 heads fully locally (no
collectives): the output conv is column-wise independent, so y[:, n-half]
only needs x[:, n-half].

Key structure (engine-balance driven; ACT exp of 8.4M scores ~72us is the
compute wall, bias HBM stream ~16 MiB is the memory wall):
  - softmax(s + b) = exp(s) * exp(b) with exp(b) precomputed on the HOST in
    bf16: halves bias HBM traffic and turns the bias add into 16-bit
    multiplies, split DVE (half A) / GpSimd (half B) -- tensor_tensor never
    contends with the shared DVE/GpSimd SBUF port pair.
  - scores psum from K=4 matmuls packed 4-concurrent via tile_position row
    groups (Kp2/Qp2 replicated at partitions 32*rg).
  - K-projection evacuates psum via full-partition [128,1024] copies into a
    staging tile; the 4-partition consolidation + row-group replication is
    16 HWDGE sbuf->sbuf DMAs (4-lane DVE copies and whole-row replication
    DMAs are both ~27 GB/s engine-pinned; this splits and overlaps them).
  - attn@V lhsT has ones in cols 0..3 and V in cols 32..35 (M=36): softmax
    denominator lands on psum partitions 0..3 and x on 32..35, both legal
    32-aligned engine AP bases -> normalization is pure DVE.
  - BN affine precomputed on host; input qkv+weights packed fp16, one
    contiguous run per partition per DMA.
"""
import sys

if "/opt/trn_rl_repo" not in sys.path:
    sys.path.insert(0, "/opt/trn_rl_repo")

import numpy as np
import ml_dtypes

import concourse.bass as bass
import concourse.tile as tile
from concourse import bacc, mybir
from concourse.bass_utils import run_bass_kernel_spmd

F32 = mybir.dt.float32
F32R = mybir.dt.float32r
BF16 = mybir.dt.bfloat16
FP16 = mybir.dt.float16
AF = mybir.ActivationFunctionType
ALU = mybir.AluOpType
PSUM = bass.MemorySpace.PSUM

H = 16
D = 4
HID = 256
B = 4
N = 1024
NH = 512          # per-core query positions
NCORES = 8
SCALE = float(D) ** -0.5
BN_EPS = 1e-5
NEG_SLOPE = 0.2

# packed input column offsets (fp16 elements): [k | w | q | v]
KOFF = 0
WOFF = 2 * N
QOFF = 2 * N + 320
VOFF = 4 * N + 320
QKVW_W = 6 * N + 320


def _emit(nc, tc, io):
    qkvw, expbT = io["qkvw"], io["expbT"]
    woT, bnst, y = io["woT"], io["bnst"], io["y"]

    with (
        tc.tile_pool(name="persist", bufs=1) as persist,
        tc.tile_pool(name="expb", bufs=3) as bp,
        tc.tile_pool(name="exps", bufs=2) as ep,
        tc.tile_pool(name="attn", bufs=2) as ap,
        tc.tile_pool(name="sml", bufs=4) as sp,
        tc.tile_pool(name="p1", bufs=1) as p1,
        tc.tile_pool(name="ps_s", bufs=2, space=PSUM) as pss,
        tc.tile_pool(name="ps_x", bufs=2, space=PSUM) as psx,
    ):
        Kst = persist.tile([128, 4096], BF16, tag="Kst")
        Kp2 = persist.tile([100, H * N], BF16, tag="Kp2")
        Qp2 = persist.tile([100, H * NH], BF16, tag="Qp2")
        Vtm = persist.tile([128, H * 8 * 8], BF16, tag="Vtm")
        xTs = persist.tile([128, 256], FP16, tag="xTs")
        x_sb = persist.tile([64, NH], F32R, tag="x_sb")
        woT_sb = persist.tile([64, HID], F32R, tag="woT_sb")
        bn_sb = persist.tile([128, 4], F32, tag="bn_sb")
        ident = persist.tile([128, 128], FP16, tag="ident")

        # ---------------- phase 0: input DMAs ----------------
        # k+weights first (K projection starts earliest), then q, then v;
        # all on the scalar HWDGE queue, one contiguous run per partition.
        qw_sb = p1.tile([128, QKVW_W], FP16, tag="qw_sb")
        nc.sync.dma_start(qw_sb[:, KOFF:QOFF], qkvw[:, KOFF:QOFF])
        nc.scalar.dma_start(qw_sb[:, QOFF:VOFF], qkvw[:, QOFF:VOFF])
        nc.scalar.dma_start(qw_sb[:, VOFF:QKVW_W], qkvw[:, VOFF:QKVW_W])
        k_sb = qw_sb[:, KOFF:KOFF + 2 * N]
        q_sb = qw_sb[:, QOFF:QOFF + 2 * N]
        v_sb = qw_sb[:, VOFF:VOFF + 2 * N]

        nc.gpsimd.dma_start(woT_sb[:], woT)
        nc.scalar.dma_start(bn_sb[:], bnst)
        nc.scalar.dma_start(ident[:], io["ident"])

        # exp-bias prefetch: pairs 0-2 up front (3-deep pool); pairs 3-7 are
        # emitted inside the head loop (after the gpsimd mults that free the
        # pool slot, so the SWDGE WAR wait cannot deadlock the gpsimd stream).
        expb_tiles = []

        def fetch_pair(pair):
            bh2 = bp.tile([128, 2 * 8 * NH], BF16, tag="bh2")
            nc.gpsimd.dma_start(
                bh2[:].rearrange("p (h t n) -> p h t n", h=2, t=8),
                expbT[pair])
            expb_tiles.append(bh2)

        for pair in range(3):
            fetch_pair(pair)

        # ---------------- phase 1: projections ----------------
        # K proj: 4 j-values col-tiled per [128,1024] psum tile (rows 32g+d
        # hold j = 4*b4+g). Evacuate each b4 with ONE full-partition copy into
        # Kst[:, 1024*b4]; consolidation into the Kp2 row groups is 16 HWDGE
        # DMAs (row group rr only ever reads g = j%4 in {0,1} for even rr /
        # {2,3} for odd rr: Kp2[32rr+d, 1024*(4b4+g)+n] = Kst[32g+d,
        # 1024b4+n]), issued per b4-half so head 0 unblocks early. Queue
        # order matches data-ready order (HWDGE is FIFO per queue).
        KpV = Kp2[:].rearrange("p (b4 g c) -> p b4 g c", b4=4, g=4)
        KsV = Kst[:].rearrange("p (b4 c) -> p b4 c", b4=4)

        def k_proj(b4):
            psk = pss.tile([128, 1024], F32, tag="ps")
            for g in range(4):
                j = 4 * b4 + g
                for nn2 in range(2):
                    for c in range(2):
                        nc.tensor.matmul(
                            psk[32 * g:32 * g + 4, 512 * nn2:512 * nn2 + 512],
                            qw_sb[:, WOFF + 160 * c + j:WOFF + 160 * c + j + 49:16],
                            k_sb[:, 1024 * c + 512 * nn2:1024 * c + 512 * nn2 + 512],
                            start=(c == 0), stop=(c == 1), tile_position=(0, 32 * g))
            if b4 % 2 == 0:
                nc.vector.tensor_copy(Kst[:, 1024 * b4:1024 * b4 + 1024], psk[:])
            else:
                nc.scalar.copy(Kst[:, 1024 * b4:1024 * b4 + 1024], psk[:])

        def cons_half(bh):
            for rr in range(4):
                for g in (0, 1) if rr % 2 == 0 else (2, 3):
                    nc.sync.dma_start(
                        KpV[32 * rr:32 * rr + 4, 2 * bh:2 * bh + 2, g, :],
                        KsV[32 * g:32 * g + 4, 2 * bh:2 * bh + 2, :])

        k_proj(0)
        k_proj(1)
        cons_half(0)

        # Q proj: same col-tiling; epilogue copies reorder to head-major Qp2
        # (scaled by SCALE), then 3 row-group replication DMAs.
        for b4 in range(2):
            psq = pss.tile([128, 1024], F32, tag="ps")
            for g in range(4):
                j = 4 * b4 + g
                for nn2 in range(2):
                    for c in range(2):
                        nc.tensor.matmul(
                            psq[32 * g:32 * g + 4, 512 * nn2:512 * nn2 + 512],
                            qw_sb[:, WOFF + 160 * c + 128 + 4 * j:
                                  WOFF + 160 * c + 128 + 4 * j + 4],
                            q_sb[:, 1024 * c + 512 * nn2:1024 * c + 512 * nn2 + 512],
                            start=(c == 0), stop=(c == 1), tile_position=(0, 32 * g))
            for g in range(4):
                j = 4 * b4 + g
                srcv = psq[32 * g:32 * g + 4, :].rearrange("d (a b) -> d b a", b=16)
                dstv = Qp2[0:4, :].rearrange("d (b q) -> d b q", b=16)[:, :, 64 * j:64 * j + 64]
                if g % 2 == 0:
                    nc.vector.tensor_scalar_mul(dstv, srcv, SCALE)
                else:
                    nc.scalar.mul(dstv, srcv, SCALE)
        for rep in range(1, 4):
            nc.sync.dma_start(Qp2[32 * rep:32 * rep + 4, :], Qp2[0:4, :])

        k_proj(2)
        k_proj(3)
        cons_half(1)

        # V projection into Vtm [128, (h, t, c8)] bf16: c 0..3 = 1.0 (the 4
        # ones columns make the softmax denominator land 4-fold replicated in
        # psum free cols 0..3), c = 4+d holds vh[m = 128t + p, d] for head h.
        # Two heads per psum tile halve the (strided, overhead-bound)
        # evacuation copy count.
        nc.vector.memset(
            Vtm[:].rearrange("p (h t c) -> p h t c", t=8, c=8)[:, :, :, 0:4], 1.0)
        for s in range(0, 16, 2):
            psv = psx.tile([64, 128], F32, tag="pn")
            for s2 in range(2):
                for c in range(2):
                    nc.tensor.matmul(
                        psv[:, 64 * s2:64 * s2 + 64],
                        v_sb[:, 1024 * c + s + s2:1024 * c + s + s2 + 1009:16],
                        qw_sb[:, WOFF + 160 * c + 64:WOFF + 160 * c + 128],
                        start=(c == 0), stop=(c == 1),
                    )
            pv = psv[:].rearrange("r (s2 d c2) -> r s2 d c2", s2=2, c2=16)
            dst = Vtm[:].rearrange("p (h t c) -> p h t c", t=8, c=8)
            nc.vector.tensor_copy(dst[0:64, s:s + 2, :, 4:8],
                                  pv[:, :, :, 0:16:2].transpose([0, 1, 3, 2]))
            nc.vector.tensor_copy(dst[64:128, s:s + 2, :, 4:8],
                                  pv[:, :, :, 1:16:2].transpose([0, 1, 3, 2]))

        # ---------------- phase 2: attention ----------------
        # scores: per m-chunk t one M=128 K=4 matmul at tile row 32*(t%4);
        # four consecutive t land on four distinct PE quadrants -> concurrent
        Kv = [Kp2[32 * rg:32 * rg + 4, :].rearrange("d (m s) -> d m s", s=16)
              for rg in range(4)]
        Qv = [Qp2[32 * rg:32 * rg + 4, :] for rg in range(4)]
        for h in range(H):
            bh2 = expb_tiles[h // 2]
            hb = 4096 * (h % 2)
            es = ep.tile([128, 4096], BF16, tag="es")
            for u in range(4):   # pairs of m-chunks -> one 2-bank psum tile
                ps = pss.tile([128, 1024], F32, tag="ps")
                for v2 in range(2):
                    t = 2 * u + v2
                    rg = t % 4
                    nc.tensor.matmul(ps[:, 512 * v2:512 * v2 + 512],
                                     Kv[rg][:, 128 * t:128 * t + 128, h],
                                     Qv[rg][:, 512 * h:512 * h + 512],
                                     start=True, stop=True,
                                     tile_position=(32 * rg, 0))
                nc.scalar.activation(es[:, 1024 * u:1024 * u + 1024], ps[:], AF.Exp)
            at = ap.tile([128, 4096], BF16, tag="at")
            nc.vector.tensor_mul(at[:, 0:3072], es[:, 0:3072], bh2[:, hb:hb + 3072])
            nc.gpsimd.tensor_mul(at[:, 3072:4096], es[:, 3072:4096],
                                 bh2[:, hb + 3072:hb + 4096])
            # attn@V, flipped: the attn [128m, 128n] chunk is the stationary
            # operand (contiguous bf16 128-col weight loads -> FWL) and only
            # the 8 Vtm columns stream. Output lands n-on-partitions: psum
            # [128n, (denominator x4 | x x4)] per n-block, two n-blocks per
            # 2-bank psum tile (cols 0.. and 512.. -> separate banks, so the
            # per-block start=True bank-clears don't interact).
            pns = []
            for half in range(2):
                pn = psx.tile([128, 1024], F32, tag="pn")
                for t in range(8):
                    for nb2 in range(2):
                        nb = 2 * half + nb2
                        nc.tensor.matmul(
                            pn[:, 512 * nb2:512 * nb2 + 8],
                            at[:, 512 * t + 128 * nb:512 * t + 128 * nb + 128],
                            Vtm[:, 64 * h + 8 * t:64 * h + 8 * t + 8],
                            start=(t == 0), stop=(t == 7))
                pns.append(pn)
            # normalize: cols 0..3 hold the denominator replicated 4x, so one
            # reciprocal + one multiply per psum tile cover 2 n-blocks
            xv = xTs[:].rearrange("p (nb c) -> p nb c", nb=4)
            for half, pn in enumerate(pns):
                pv8 = pn[:].rearrange("p (nb c) -> p nb c", nb=2)
                rec = sp.tile([128, 8], F32, tag="rec")
                rv = rec[:].rearrange("p (nb c) -> p nb c", nb=2)
                nc.vector.reciprocal_approx_fast(rv, pv8[:, :, 0:4])
                nc.vector.tensor_mul(
                    xv[:, 2 * half:2 * half + 2, 4 * h:4 * h + 4],
                    pv8[:, :, 4:8], rv)
            # fetch the next exp-bias pair once the gpsimd mult above has
            # freed its pool slot (keeps the WAR wait behind the consumers
            # in the gpsimd instruction stream)
            if h % 2 == 1 and 3 + (h - 1) // 2 < 8:
                fetch_pair(3 + (h - 1) // 2)

        # ---------------- phase 3: transpose x, output conv + BN + LeakyReLU ----
        # xTs [128n, (nb, c)] -> x_sb [64c, 512n] via 4 PE transposes
        for nb in range(4):
            pt = pss.tile([64, 128], FP16, tag="ps")
            nc.tensor.transpose(pt[:], xTs[:, 64 * nb:64 * nb + 64], ident[:])
            nc.vector.tensor_copy(x_sb[:, 128 * nb:128 * nb + 128], pt[:])
        for u in range(2):
            psy = pss.tile([128, NH], F32, tag="ps")
            nc.tensor.matmul(psy[:], woT_sb[0:64, 128 * u:128 * u + 128], x_sb[:],
                             start=True, stop=True)
            y2 = sp.tile([128, NH], F32, tag="y2")
            nc.vector.tensor_scalar(y2[:], psy[:], bn_sb[:, u:u + 1], bn_sb[:, 2 + u:3 + u],
                                    ALU.mult, ALU.add)
            yt = sp.tile([128, NH], F32, tag="yt")
            nc.vector.scalar_tensor_tensor(yt[:], y2[:], NEG_SLOPE, y2[:],
                                           ALU.mult, ALU.max)
            nc.sync.dma_start(y[u], yt[:])


def build_program():
    nc = bacc.Bacc("TRN2", target_bir_lowering=False, debug=False)
    io = {
        "qkvw": nc.dram_tensor("qkvw", [128, QKVW_W], FP16,
                               kind="ExternalInput").ap(),
        "expbT": nc.dram_tensor("expbT", [8, 128, 2, 8, NH], BF16,
                                kind="ExternalInput").ap(),
        "woT": nc.dram_tensor("woT", [64, HID], F32, kind="ExternalInput").ap(),
        "bnst": nc.dram_tensor("bnst", [128, 4], F32, kind="ExternalInput").ap(),
        "ident": nc.dram_tensor("ident", [128, 128], FP16, kind="ExternalInput").ap(),
        "y": nc.dram_tensor("y", [2, 128, NH], F32, kind="ExternalOutput").ap(),
    }
    with tile.TileContext(nc) as tc:
        _emit(nc, tc, io)
    nc.compile()
    return nc


def make_in_maps(q, k, v, attn_bias, Wq, Wk, Wv, Wo, bo, gamma, beta, run_mean, run_var):
    def f32(x):
        return np.ascontiguousarray(np.asarray(x, dtype=np.float32))

    bf16 = ml_dtypes.bfloat16
    q, k, v = f32(q), f32(k), f32(v)
    Wq, Wk, Wv, Wo, bo = f32(Wq), f32(Wk), f32(Wv), f32(Wo), f32(bo)
    gamma, beta, run_mean, run_var = f32(gamma), f32(beta), f32(run_mean), f32(run_var)

    woT = f32(Wo.T)
    # BN affine precomputed on host: s = gamma*rsqrt(var+eps),
    # t = (bo - mean)*s + beta; packed [128, (s0,s1,t0,t1)]
    s = gamma / np.sqrt(run_var + BN_EPS)
    t = (bo - run_mean) * s + beta
    bnst = f32(np.concatenate([s.reshape(2, 128).T, t.reshape(2, 128).T], axis=1))

    wkT16 = Wk.T.astype(np.float16)     # [256, 64]
    wvT16 = Wv.T.astype(np.float16)

    # exp of the additive bias, in bf16 (softmax(s+b) = exp(s)*exp(b))
    expb = np.exp(np.asarray(attn_bias, dtype=np.float32)).astype(bf16)

    def fold(a, width):
        return a.reshape(2, 128, width).transpose(1, 0, 2).reshape(128, 2 * width)

    in_maps = []
    for core in range(NCORES):
        b, half = divmod(core, 2)
        n0 = half * NH
        rows = np.array([16 * d + 8 * half + jl for jl in range(8) for d in range(4)])
        wqT16 = Wq[rows, :].T.astype(np.float16)                  # [256, 32]
        # packed [128p, (k | wk,wv,wq | q | v)], fp16, c2-folded per partition
        qkvw = np.empty((128, QKVW_W), dtype=np.float16)
        qkvw[:, KOFF:KOFF + 2 * N] = fold(k[b], N)
        wcat = np.concatenate([wkT16, wvT16, wqT16], axis=1)      # [256, 160]
        qkvw[:, WOFF:WOFF + 320] = fold(wcat, 160)
        qkvw[:, QOFF:QOFF + 2 * N] = fold(q[b], N)
        qkvw[:, VOFF:VOFF + 2 * N] = fold(v[b], N)

        # expbT [pair, p, h2, t, n]: exp(bias)[m = 128t + p, n-half]
        bt = expb[b, :, n0:n0 + NH, :].transpose(0, 2, 1)         # [16h, 1024m, 512n]
        expbT = np.ascontiguousarray(
            bt.reshape(8, 2, 8, 128, NH).transpose(0, 3, 1, 2, 4))
        in_maps.append({
            "qkvw": np.ascontiguousarray(qkvw),
            "expbT": expbT,
            "woT": woT, "bnst": bnst,
            "ident": np.eye(128, dtype=np.float16),
        })
    return in_maps


_NC_CACHE = None


def get_nc():
    global _NC_CACHE
    if _NC_CACHE is None:
        _NC_CACHE = build_program()
    return _NC_CACHE


def kernel(**inputs):
    nc = get_nc()
    in_maps = make_in_maps(**inputs)
    res = run_bass_kernel_spmd(nc, in_maps, list(range(NCORES)))
    out = np.empty((B, HID, N), dtype=np.float32)
    for core in range(NCORES):
        b, half = divmod(core, 2)
        out[b, :, half * NH:(half + 1) * NH] = \
            res.results[core]["y"].reshape(HID, NH)
    return out


# revision 40
# speedup vs baseline: 1.0949x; 1.0949x over previous
"""Trainium2 Bass kernel for nn_MultiHeadAttention_80418967650946.

Reference computation (per batch b):
  qp/kp/vp = 1x1-conv projections of q/k/v   [64, N]
  funky head view: qh[h,n,d] = qp.reshape(4, 16*N)[d, 16n+h]  (same for kh, vh)
  scores = qh @ kh * 0.25^0.5 + bias ; attn = softmax(scores)
  x[4h+d, n] = (attn @ vh)[h, n, d] ; y = LeakyReLU(BN(Wo @ x + bo), 0.2)

Sharding: 8 cores = 4 batches x 2 query-halves (n in [0,512) or [512,1024)).
Each core computes its query-half for ALL 16 heads fully locally (no
collectives): the output conv is column-wise independent, so y[:, n-half]
only needs x[:, n-half].

Key structure (engine-balance driven; ACT exp of 8.4M scores ~72us is the
compute wall, bias HBM stream ~16 MiB is the memory wall):
  - softmax(s + b) = exp(s) * exp(b) with exp(b) precomputed on the HOST in
    bf16: halves bias HBM traffic and turns the bias add into 16-bit
    multiplies, split DVE (half A) / GpSimd (half B) -- tensor_tensor never
    contends with the shared DVE/GpSimd SBUF port pair.
  - scores psum from K=4 matmuls packed 4-concurrent via tile_position row
    groups (Kp2/Qp2 replicated at partitions 32*rg).
  - K-projection evacuates psum via full-partition [128,1024] copies into a
    staging tile; the 4-partition consolidation + row-group replication is
    16 HWDGE sbuf->sbuf DMAs (4-lane DVE copies and whole-row replication
    DMAs are both ~27 GB/s engine-pinned; this splits and overlaps them).
  - attn@V lhsT has ones in cols 0..3 and V in cols 32..35 (M=36): softmax
    denominator lands on psum partitions 0..3 and x on 32..35, both legal
    32-aligned engine AP bases -> normalization is pure DVE.
  - BN affine precomputed on host; input qkv+weights packed fp16, one
    contiguous run per partition per DMA.
"""
import sys

if "/opt/trn_rl_repo" not in sys.path:
    sys.path.insert(0, "/opt/trn_rl_repo")

import numpy as np
import ml_dtypes

import concourse.bass as bass
import concourse.tile as tile
from concourse import bacc, mybir
from concourse.bass_utils import run_bass_kernel_spmd

F32 = mybir.dt.float32
F32R = mybir.dt.float32r
BF16 = mybir.dt.bfloat16
FP16 = mybir.dt.float16
AF = mybir.ActivationFunctionType
ALU = mybir.AluOpType
PSUM = bass.MemorySpace.PSUM

H = 16
D = 4
HID = 256
B = 4
N = 1024
NH = 512          # per-core query positions
NCORES = 8
SCALE = float(D) ** -0.5
BN_EPS = 1e-5
NEG_SLOPE = 0.2

# packed input column offsets (fp16 elements): [k | w | q | v]
KOFF = 0
WOFF = 2 * N
QOFF = 2 * N + 320
VOFF = 4 * N + 320
QKVW_W = 6 * N + 320


def _emit(nc, tc, io):
    qkvw, expbT = io["qkvw"], io["expbT"]
    woT, bnst, y = io["woT"], io["bnst"], io["y"]

    with (
        tc.tile_pool(name="persist", bufs=1) as persist,
        tc.tile_pool(name="expb", bufs=4) as bp,
        tc.tile_pool(name="exps", bufs=2) as ep,
        tc.tile_pool(name="attn", bufs=2) as ap,
        tc.tile_pool(name="sml", bufs=4) as sp,
        tc.tile_pool(name="p1", bufs=1) as p1,
        tc.tile_pool(name="ps_s", bufs=3, space=PSUM) as pss,
        tc.tile_pool(name="ps_x", bufs=2, space=PSUM) as psx,
    ):
        Kst = persist.tile([128, 4096], BF16, tag="Kst")
        Kp2 = persist.tile([100, H * N], BF16, tag="Kp2")
        Qp2 = persist.tile([100, H * NH], BF16, tag="Qp2")
        Vtm = persist.tile([128, H * 8 * 8], BF16, tag="Vtm")
        xTs = persist.tile([128, 256], FP16, tag="xTs")
        x_sb = persist.tile([64, NH], F32R, tag="x_sb")
        woT_sb = persist.tile([64, HID], F32R, tag="woT_sb")
        bn_sb = persist.tile([128, 4], F32, tag="bn_sb")
        ident = persist.tile([128, 128], FP16, tag="ident")

        # ---------------- phase 0: input DMAs ----------------
        # k+weights first (K projection starts earliest), then q, then v;
        # all on the scalar HWDGE queue, one contiguous run per partition.
        qw_sb = p1.tile([128, QKVW_W], FP16, tag="qw_sb")
        nc.sync.dma_start(qw_sb[:, KOFF:QOFF], qkvw[:, KOFF:QOFF])
        nc.scalar.dma_start(qw_sb[:, QOFF:VOFF], qkvw[:, QOFF:VOFF])
        nc.scalar.dma_start(qw_sb[:, VOFF:QKVW_W], qkvw[:, VOFF:QKVW_W])
        k_sb = qw_sb[:, KOFF:KOFF + 2 * N]
        q_sb = qw_sb[:, QOFF:QOFF + 2 * N]
        v_sb = qw_sb[:, VOFF:VOFF + 2 * N]

        nc.gpsimd.dma_start(woT_sb[:], woT)
        nc.scalar.dma_start(bn_sb[:], bnst)
        nc.scalar.dma_start(ident[:], io["ident"])

        # exp-bias prefetch: pairs 0-2 up front (3-deep pool); pairs 3-7 are
        # emitted inside the head loop (after the gpsimd mults that free the
        # pool slot, so the SWDGE WAR wait cannot deadlock the gpsimd stream).
        expb_tiles = []

        def fetch_pair(pair):
            bh2 = bp.tile([128, 2 * 8 * NH], BF16, tag="bh2")
            nc.gpsimd.dma_start(
                bh2[:].rearrange("p (h t n) -> p h t n", h=2, t=8),
                expbT[pair])
            expb_tiles.append(bh2)

        for pair in range(4):
            fetch_pair(pair)

        # ---------------- phase 1: projections ----------------
        # K proj: 4 j-values col-tiled per [128,1024] psum tile (rows 32g+d
        # hold j = 4*b4+g). Evacuate each b4 with ONE full-partition copy into
        # Kst[:, 1024*b4]; consolidation into the Kp2 row groups is 16 HWDGE
        # DMAs (row group rr only ever reads g = j%4 in {0,1} for even rr /
        # {2,3} for odd rr: Kp2[32rr+d, 1024*(4b4+g)+n] = Kst[32g+d,
        # 1024b4+n]), issued per b4-half so head 0 unblocks early. Queue
        # order matches data-ready order (HWDGE is FIFO per queue).
        KpV = Kp2[:].rearrange("p (b4 g c) -> p b4 g c", b4=4, g=4)
        KsV = Kst[:].rearrange("p (b4 c) -> p b4 c", b4=4)

        def k_proj(b4):
            psk = pss.tile([128, 1024], F32, tag="ps")
            for g in range(4):
                j = 4 * b4 + g
                for nn2 in range(2):
                    for c in range(2):
                        nc.tensor.matmul(
                            psk[32 * g:32 * g + 4, 512 * nn2:512 * nn2 + 512],
                            qw_sb[:, WOFF + 160 * c + j:WOFF + 160 * c + j + 49:16],
                            k_sb[:, 1024 * c + 512 * nn2:1024 * c + 512 * nn2 + 512],
                            start=(c == 0), stop=(c == 1), tile_position=(0, 32 * g))
            if b4 % 2 == 0:
                nc.vector.tensor_copy(Kst[:, 1024 * b4:1024 * b4 + 1024], psk[:])
            else:
                nc.scalar.copy(Kst[:, 1024 * b4:1024 * b4 + 1024], psk[:])

        def cons_half(bh):
            for rr in range(4):
                for g in (0, 1) if rr % 2 == 0 else (2, 3):
                    nc.sync.dma_start(
                        KpV[32 * rr:32 * rr + 4, 2 * bh:2 * bh + 2, g, :],
                        KsV[32 * g:32 * g + 4, 2 * bh:2 * bh + 2, :])

        k_proj(0)
        k_proj(1)
        cons_half(0)

        # Q proj: same col-tiling; epilogue copies reorder to head-major Qp2
        # (scaled by SCALE), then 3 row-group replication DMAs.
        for b4 in range(2):
            psq = pss.tile([128, 1024], F32, tag="ps")
            for g in range(4):
                j = 4 * b4 + g
                for nn2 in range(2):
                    for c in range(2):
                        nc.tensor.matmul(
                            psq[32 * g:32 * g + 4, 512 * nn2:512 * nn2 + 512],
                            qw_sb[:, WOFF + 160 * c + 128 + 4 * j:
                                  WOFF + 160 * c + 128 + 4 * j + 4],
                            q_sb[:, 1024 * c + 512 * nn2:1024 * c + 512 * nn2 + 512],
                            start=(c == 0), stop=(c == 1), tile_position=(0, 32 * g))
            for g in range(4):
                j = 4 * b4 + g
                srcv = psq[32 * g:32 * g + 4, :].rearrange("d (a b) -> d b a", b=16)
                dstv = Qp2[0:4, :].rearrange("d (b q) -> d b q", b=16)[:, :, 64 * j:64 * j + 64]
                if g % 2 == 0:
                    nc.vector.tensor_scalar_mul(dstv, srcv, SCALE)
                else:
                    nc.scalar.mul(dstv, srcv, SCALE)
        for rep in range(1, 4):
            nc.sync.dma_start(Qp2[32 * rep:32 * rep + 4, :], Qp2[0:4, :])

        k_proj(2)
        k_proj(3)
        cons_half(1)

        # V projection into Vtm [128, (h, t, c8)] bf16: c 0..3 = 1.0 (the 4
        # ones columns make the softmax denominator land 4-fold replicated in
        # psum free cols 0..3), c = 4+d holds vh[m = 128t + p, d] for head h.
        # Two heads per psum tile halve the (strided, overhead-bound)
        # evacuation copy count.
        nc.vector.memset(
            Vtm[:].rearrange("p (h t c) -> p h t c", t=8, c=8)[:, :, :, 0:4], 1.0)
        for s in range(0, 16, 2):
            psv = psx.tile([64, 128], F32, tag="pn")
            for s2 in range(2):
                for c in range(2):
                    nc.tensor.matmul(
                        psv[:, 64 * s2:64 * s2 + 64],
                        v_sb[:, 1024 * c + s + s2:1024 * c + s + s2 + 1009:16],
                        qw_sb[:, WOFF + 160 * c + 64:WOFF + 160 * c + 128],
                        start=(c == 0), stop=(c == 1),
                    )
            pv = psv[:].rearrange("r (s2 d c2) -> r s2 d c2", s2=2, c2=16)
            dst = Vtm[:].rearrange("p (h t c) -> p h t c", t=8, c=8)
            nc.vector.tensor_copy(dst[0:64, s:s + 2, :, 4:8],
                                  pv[:, :, :, 0:16:2].transpose([0, 1, 3, 2]))
            nc.vector.tensor_copy(dst[64:128, s:s + 2, :, 4:8],
                                  pv[:, :, :, 1:16:2].transpose([0, 1, 3, 2]))

        # ---------------- phase 2: attention ----------------
        # scores: per m-chunk t one M=128 K=4 matmul at tile row 32*(t%4);
        # four consecutive t land on four distinct PE quadrants -> concurrent
        Kv = [Kp2[32 * rg:32 * rg + 4, :].rearrange("d (m s) -> d m s", s=16)
              for rg in range(4)]
        Qv = [Qp2[32 * rg:32 * rg + 4, :] for rg in range(4)]
        for h in range(H):
            bh2 = expb_tiles[h // 2]
            hb = 4096 * (h % 2)
            es = ep.tile([128, 4096], BF16, tag="es")
            for u in range(4):   # pairs of m-chunks -> one 2-bank psum tile
                ps = pss.tile([128, 1024], F32, tag="ps")
                for v2 in range(2):
                    t = 2 * u + v2
                    rg = t % 4
                    nc.tensor.matmul(ps[:, 512 * v2:512 * v2 + 512],
                                     Kv[rg][:, 128 * t:128 * t + 128, h],
                                     Qv[rg][:, 512 * h:512 * h + 512],
                                     start=True, stop=True,
                                     tile_position=(32 * rg, 0))
                nc.scalar.activation(es[:, 1024 * u:1024 * u + 1024], ps[:], AF.Exp)
            at = ap.tile([128, 4096], BF16, tag="at")
            nc.vector.tensor_mul(at[:, 0:3072], es[:, 0:3072], bh2[:, hb:hb + 3072])
            nc.gpsimd.tensor_mul(at[:, 3072:4096], es[:, 3072:4096],
                                 bh2[:, hb + 3072:hb + 4096])
            # attn@V, flipped: the attn [128m, 128n] chunk is the stationary
            # operand (contiguous bf16 128-col weight loads -> FWL) and only
            # the 8 Vtm columns stream. Output lands n-on-partitions: psum
            # [128n, (denominator x4 | x x4)] per n-block, two n-blocks per
            # 2-bank psum tile (cols 0.. and 512.. -> separate banks, so the
            # per-block start=True bank-clears don't interact).
            # Both n-blocks of a half share ONE psum bank (cols 0..7 / 8..15):
            # the first matmul's start=True clears the whole bank's
            # has_written bits, so the second block's t=0 (start=False)
            # overwrites rather than accumulating stale data -- PE executes
            # in order, making this deterministic.
            pns = []
            for half in range(2):
                pn = psx.tile([128, 512], F32, tag="pn")
                for t in range(8):
                    for nb2 in range(2):
                        nb = 2 * half + nb2
                        nc.tensor.matmul(
                            pn[:, 8 * nb2:8 * nb2 + 8],
                            at[:, 512 * t + 128 * nb:512 * t + 128 * nb + 128],
                            Vtm[:, 64 * h + 8 * t:64 * h + 8 * t + 8],
                            start=(t == 0 and nb2 == 0), stop=(t == 7),
                            skip_group_check=True)
                pns.append(pn)
            # normalize: cols 0..3 hold the denominator replicated 4x, so one
            # reciprocal + one multiply per psum tile cover 2 n-blocks
            xv = xTs[:].rearrange("p (nb c) -> p nb c", nb=4)
            for half, pn in enumerate(pns):
                pv8 = pn[:, 0:16].rearrange("p (nb c) -> p nb c", nb=2)
                rec = sp.tile([128, 8], F32, tag="rec")
                rv = rec[:].rearrange("p (nb c) -> p nb c", nb=2)
                nc.vector.reciprocal_approx_fast(rv, pv8[:, :, 0:4])
                nc.vector.tensor_mul(
                    xv[:, 2 * half:2 * half + 2, 4 * h:4 * h + 4],
                    pv8[:, :, 4:8], rv)
            # fetch the next exp-bias pair once the gpsimd mult above has
            # freed its pool slot (keeps the WAR wait behind the consumers
            # in the gpsimd instruction stream)
            if h % 2 == 1 and 4 + (h - 1) // 2 < 8:
                fetch_pair(4 + (h - 1) // 2)

        # ---------------- phase 3: transpose x, output conv + BN + LeakyReLU ----
        # xTs [128n, (nb, c)] -> x_sb [64c, 512n] via 4 PE transposes
        for nb in range(4):
            pt = pss.tile([64, 128], FP16, tag="ps")
            nc.tensor.transpose(pt[:], xTs[:, 64 * nb:64 * nb + 64], ident[:])
            nc.vector.tensor_copy(x_sb[:, 128 * nb:128 * nb + 128], pt[:])
        for u in range(2):
            psy = pss.tile([128, NH], F32, tag="ps")
            nc.tensor.matmul(psy[:], woT_sb[0:64, 128 * u:128 * u + 128], x_sb[:],
                             start=True, stop=True)
            y2 = sp.tile([128, NH], F32, tag="y2")
            nc.vector.tensor_scalar(y2[:], psy[:], bn_sb[:, u:u + 1], bn_sb[:, 2 + u:3 + u],
                                    ALU.mult, ALU.add)
            yt = sp.tile([128, NH], F32, tag="yt")
            nc.vector.scalar_tensor_tensor(yt[:], y2[:], NEG_SLOPE, y2[:],
                                           ALU.mult, ALU.max)
            nc.sync.dma_start(y[u], yt[:])


def build_program():
    nc = bacc.Bacc("TRN2", target_bir_lowering=False, debug=False)
    io = {
        "qkvw": nc.dram_tensor("qkvw", [128, QKVW_W], FP16,
                               kind="ExternalInput").ap(),
        "expbT": nc.dram_tensor("expbT", [8, 128, 2, 8, NH], BF16,
                                kind="ExternalInput").ap(),
        "woT": nc.dram_tensor("woT", [64, HID], F32, kind="ExternalInput").ap(),
        "bnst": nc.dram_tensor("bnst", [128, 4], F32, kind="ExternalInput").ap(),
        "ident": nc.dram_tensor("ident", [128, 128], FP16, kind="ExternalInput").ap(),
        "y": nc.dram_tensor("y", [2, 128, NH], F32, kind="ExternalOutput").ap(),
    }
    with tile.TileContext(nc) as tc:
        _emit(nc, tc, io)
    nc.compile()
    return nc


def make_in_maps(q, k, v, attn_bias, Wq, Wk, Wv, Wo, bo, gamma, beta, run_mean, run_var):
    def f32(x):
        return np.ascontiguousarray(np.asarray(x, dtype=np.float32))

    bf16 = ml_dtypes.bfloat16
    q, k, v = f32(q), f32(k), f32(v)
    Wq, Wk, Wv, Wo, bo = f32(Wq), f32(Wk), f32(Wv), f32(Wo), f32(bo)
    gamma, beta, run_mean, run_var = f32(gamma), f32(beta), f32(run_mean), f32(run_var)

    woT = f32(Wo.T)
    # BN affine precomputed on host: s = gamma*rsqrt(var+eps),
    # t = (bo - mean)*s + beta; packed [128, (s0,s1,t0,t1)]
    s = gamma / np.sqrt(run_var + BN_EPS)
    t = (bo - run_mean) * s + beta
    bnst = f32(np.concatenate([s.reshape(2, 128).T, t.reshape(2, 128).T], axis=1))

    wkT16 = Wk.T.astype(np.float16)     # [256, 64]
    wvT16 = Wv.T.astype(np.float16)

    # exp of the additive bias, in bf16 (softmax(s+b) = exp(s)*exp(b))
    expb = np.exp(np.asarray(attn_bias, dtype=np.float32)).astype(bf16)

    def fold(a, width):
        return a.reshape(2, 128, width).transpose(1, 0, 2).reshape(128, 2 * width)

    in_maps = []
    for core in range(NCORES):
        b, half = divmod(core, 2)
        n0 = half * NH
        rows = np.array([16 * d + 8 * half + jl for jl in range(8) for d in range(4)])
        wqT16 = Wq[rows, :].T.astype(np.float16)                  # [256, 32]
        # packed [128p, (k | wk,wv,wq | q | v)], fp16, c2-folded per partition
        qkvw = np.empty((128, QKVW_W), dtype=np.float16)
        qkvw[:, KOFF:KOFF + 2 * N] = fold(k[b], N)
        wcat = np.concatenate([wkT16, wvT16, wqT16], axis=1)      # [256, 160]
        qkvw[:, WOFF:WOFF + 320] = fold(wcat, 160)
        qkvw[:, QOFF:QOFF + 2 * N] = fold(q[b], N)
        qkvw[:, VOFF:VOFF + 2 * N] = fold(v[b], N)

        # expbT [pair, p, h2, t, n]: exp(bias)[m = 128t + p, n-half]
        bt = expb[b, :, n0:n0 + NH, :].transpose(0, 2, 1)         # [16h, 1024m, 512n]
        expbT = np.ascontiguousarray(
            bt.reshape(8, 2, 8, 128, NH).transpose(0, 3, 1, 2, 4))
        in_maps.append({
            "qkvw": np.ascontiguousarray(qkvw),
            "expbT": expbT,
            "woT": woT, "bnst": bnst,
            "ident": np.eye(128, dtype=np.float16),
        })
    return in_maps


_NC_CACHE = None


def get_nc():
    global _NC_CACHE
    if _NC_CACHE is None:
        _NC_CACHE = build_program()
    return _NC_CACHE


def kernel(**inputs):
    nc = get_nc()
    in_maps = make_in_maps(**inputs)
    res = run_bass_kernel_spmd(nc, in_maps, list(range(NCORES)))
    out = np.empty((B, HID, N), dtype=np.float32)
    for core in range(NCORES):
        b, half = divmod(core, 2)
        out[b, :, half * NH:(half + 1) * NH] = \
            res.results[core]["y"].reshape(HID, NH)
    return out


# revision 41
# speedup vs baseline: 1.1321x; 1.0340x over previous
"""Trainium2 Bass kernel for nn_MultiHeadAttention_80418967650946.

Reference computation (per batch b):
  qp/kp/vp = 1x1-conv projections of q/k/v   [64, N]
  funky head view: qh[h,n,d] = qp.reshape(4, 16*N)[d, 16n+h]  (same for kh, vh)
  scores = qh @ kh * 0.25^0.5 + bias ; attn = softmax(scores)
  x[4h+d, n] = (attn @ vh)[h, n, d] ; y = LeakyReLU(BN(Wo @ x + bo), 0.2)

Sharding: 8 cores = 4 batches x 2 query-halves (n in [0,512) or [512,1024)).
Each core computes its query-half for ALL 16 heads fully locally (no
collectives): the output conv is column-wise independent, so y[:, n-half]
only needs x[:, n-half].

Key structure (engine-balance driven; ACT exp of 8.4M scores ~72us is the
compute wall, bias HBM stream ~16 MiB is the memory wall):
  - softmax(s + b) = exp(s) * exp(b) with exp(b) precomputed on the HOST in
    bf16: halves bias HBM traffic and turns the bias add into 16-bit
    multiplies, split DVE (half A) / GpSimd (half B) -- tensor_tensor never
    contends with the shared DVE/GpSimd SBUF port pair.
  - scores psum from K=4 matmuls packed 4-concurrent via tile_position row
    groups (Kp2/Qp2 replicated at partitions 32*rg).
  - K-projection evacuates psum via full-partition [128,1024] copies into a
    staging tile; the 4-partition consolidation + row-group replication is
    16 HWDGE sbuf->sbuf DMAs (4-lane DVE copies and whole-row replication
    DMAs are both ~27 GB/s engine-pinned; this splits and overlaps them).
  - attn@V lhsT has ones in cols 0..3 and V in cols 32..35 (M=36): softmax
    denominator lands on psum partitions 0..3 and x on 32..35, both legal
    32-aligned engine AP bases -> normalization is pure DVE.
  - BN affine precomputed on host; input qkv+weights packed fp16, one
    contiguous run per partition per DMA.
"""
import sys

if "/opt/trn_rl_repo" not in sys.path:
    sys.path.insert(0, "/opt/trn_rl_repo")

import numpy as np
import ml_dtypes

import concourse.bass as bass
import concourse.tile as tile
from concourse import bacc, mybir
from concourse.bass_utils import run_bass_kernel_spmd

F32 = mybir.dt.float32
F32R = mybir.dt.float32r
BF16 = mybir.dt.bfloat16
FP16 = mybir.dt.float16
AF = mybir.ActivationFunctionType
ALU = mybir.AluOpType
PSUM = bass.MemorySpace.PSUM

H = 16
D = 4
HID = 256
B = 4
N = 1024
NH = 512          # per-core query positions
NCORES = 8
SCALE = float(D) ** -0.5
BN_EPS = 1e-5
NEG_SLOPE = 0.2

# packed input column offsets (fp16 elements): [k | w | q | v]
KOFF = 0
WOFF = 2 * N
QOFF = 2 * N + 320
VOFF = 4 * N + 320
QKVW_W = 6 * N + 320


def _emit(nc, tc, io):
    qkvw, expbT = io["qkvw"], io["expbT"]
    woT, bnst, y = io["woT"], io["bnst"], io["y"]

    with (
        tc.tile_pool(name="persist", bufs=1) as persist,
        tc.tile_pool(name="expb", bufs=4) as bp,
        tc.tile_pool(name="exps", bufs=2) as ep,
        tc.tile_pool(name="attn", bufs=2) as ap,
        tc.tile_pool(name="sml", bufs=4) as sp,
        tc.tile_pool(name="p1", bufs=1) as p1,
        tc.tile_pool(name="ps_s", bufs=3, space=PSUM) as pss,
        tc.tile_pool(name="ps_x", bufs=2, space=PSUM) as psx,
    ):
        Kst = persist.tile([128, 4096], BF16, tag="Kst")
        Kp2 = persist.tile([100, H * N], BF16, tag="Kp2")
        Qp2 = persist.tile([100, H * NH], BF16, tag="Qp2")
        Vtm = persist.tile([128, H * 8 * 8], BF16, tag="Vtm")
        xTs = persist.tile([128, 256], FP16, tag="xTs")
        x_sb = persist.tile([64, NH], F32R, tag="x_sb")
        woT_sb = persist.tile([64, HID], F32R, tag="woT_sb")
        bn_sb = persist.tile([128, 4], F32, tag="bn_sb")
        ident = persist.tile([128, 128], FP16, tag="ident")

        # ---------------- phase 0: input DMAs ----------------
        # k+weights first (K projection starts earliest), then q, then v;
        # all on the scalar HWDGE queue, one contiguous run per partition.
        qw_sb = p1.tile([128, QKVW_W], FP16, tag="qw_sb")
        nc.sync.dma_start(qw_sb[:, KOFF:QOFF], qkvw[:, KOFF:QOFF])
        nc.scalar.dma_start(qw_sb[:, QOFF:VOFF], qkvw[:, QOFF:VOFF])
        nc.scalar.dma_start(qw_sb[:, VOFF:QKVW_W], qkvw[:, VOFF:QKVW_W])
        k_sb = qw_sb[:, KOFF:KOFF + 2 * N]
        q_sb = qw_sb[:, QOFF:QOFF + 2 * N]
        v_sb = qw_sb[:, VOFF:VOFF + 2 * N]

        nc.gpsimd.dma_start(woT_sb[:], woT)
        nc.scalar.dma_start(bn_sb[:], bnst)
        nc.scalar.dma_start(ident[:], io["ident"])

        # exp-bias prefetch on the sync HWDGE queue (no cast involved, and it
        # keeps DMA-issue work off the gpsimd stream, which now only runs
        # multiplies). Pair 0 queues first; pairs 1-3 queue after the
        # consolidation DMAs (sync HWDGE is FIFO per queue, so a 2 MiB pair
        # must not sit in front of head-0's Kp2/Qp2 pieces); pairs 4-7 are
        # emitted inside the head loop once their pool slot is free.
        expb_tiles = []

        def fetch_pair(pair):
            bh2 = bp.tile([128, 2 * 8 * NH], BF16, tag="bh2")
            nc.sync.dma_start(
                bh2[:].rearrange("p (h t n) -> p h t n", h=2, t=8),
                expbT[pair])
            expb_tiles.append(bh2)

        fetch_pair(0)

        # ---------------- phase 1: projections ----------------
        # Q proj first (its epilogue + replication chain is longest): 4
        # j-values col-tiled per [128,1024] psum tile; epilogue copies
        # reorder to head-major Qp2 (scaled by SCALE); after each b4 half, 3
        # strided row-group replication DMAs for that half (Qp2 free =
        # 512*head + 256*b4 + 64*g + a).
        for b4 in range(2):
            psq = pss.tile([128, 1024], F32, tag="ps")
            for g in range(4):
                j = 4 * b4 + g
                for nn2 in range(2):
                    for c in range(2):
                        nc.tensor.matmul(
                            psq[32 * g:32 * g + 4, 512 * nn2:512 * nn2 + 512],
                            qw_sb[:, WOFF + 160 * c + 128 + 4 * j:
                                  WOFF + 160 * c + 128 + 4 * j + 4],
                            q_sb[:, 1024 * c + 512 * nn2:1024 * c + 512 * nn2 + 512],
                            start=(c == 0), stop=(c == 1), tile_position=(0, 32 * g))
            for g in range(4):
                j = 4 * b4 + g
                srcv = psq[32 * g:32 * g + 4, :].rearrange("d (a b) -> d b a", b=16)
                dstv = Qp2[0:4, :].rearrange("d (b q) -> d b q", b=16)[:, :, 64 * j:64 * j + 64]
                if g % 2 == 0:
                    nc.vector.tensor_scalar_mul(dstv, srcv, SCALE)
                else:
                    nc.scalar.mul(dstv, srcv, SCALE)
            for rep in range(1, 4):
                nc.sync.dma_start(
                    Qp2[32 * rep:32 * rep + 4, :].rearrange(
                        "d (b hf x) -> d b hf x", b=16, hf=2)[:, :, b4, :],
                    Qp2[0:4, :].rearrange(
                        "d (b hf x) -> d b hf x", b=16, hf=2)[:, :, b4, :])

        # K proj: same col-tiling. Evacuate each b4 with ONE full-partition
        # copy into Kst[:, 1024*b4]; consolidation into the Kp2 row groups is
        # 8 HWDGE DMAs (row group rr only ever reads g = j%4 in {0,1} for
        # even rr / {2,3} for odd rr: Kp2[32rr+d, 1024*(4b4+g)+n] =
        # Kst[32g+d, 1024b4+n]).
        KpV = Kp2[:].rearrange("p (b4 g c) -> p b4 g c", b4=4, g=4)
        KsV = Kst[:].rearrange("p (b4 c) -> p b4 c", b4=4)
        for b4 in range(4):
            psk = pss.tile([128, 1024], F32, tag="ps")
            for g in range(4):
                j = 4 * b4 + g
                for nn2 in range(2):
                    for c in range(2):
                        nc.tensor.matmul(
                            psk[32 * g:32 * g + 4, 512 * nn2:512 * nn2 + 512],
                            qw_sb[:, WOFF + 160 * c + j:WOFF + 160 * c + j + 49:16],
                            k_sb[:, 1024 * c + 512 * nn2:1024 * c + 512 * nn2 + 512],
                            start=(c == 0), stop=(c == 1), tile_position=(0, 32 * g))
            if b4 % 2 == 0:
                nc.vector.tensor_copy(Kst[:, 1024 * b4:1024 * b4 + 1024], psk[:])
            else:
                nc.scalar.copy(Kst[:, 1024 * b4:1024 * b4 + 1024], psk[:])
        for rr in range(4):
            for g in (0, 1) if rr % 2 == 0 else (2, 3):
                nc.sync.dma_start(KpV[32 * rr:32 * rr + 4, :, g, :],
                                  KsV[32 * g:32 * g + 4, :, :])
        for pair in range(1, 4):
            fetch_pair(pair)

        # V projection into Vtm [128, (h, t, c8)] bf16: c 0..3 = 1.0 (the 4
        # ones columns make the softmax denominator land 4-fold replicated in
        # psum free cols 0..3), c = 4+d holds vh[m = 128t + p, d] for head h.
        # Two heads per psum tile halve the (strided, overhead-bound)
        # evacuation copy count.
        nc.vector.memset(
            Vtm[:].rearrange("p (h t c) -> p h t c", t=8, c=8)[:, :, :, 0:4], 1.0)
        for s in range(0, 16, 2):
            psv = psx.tile([64, 128], F32, tag="pn")
            for s2 in range(2):
                for c in range(2):
                    nc.tensor.matmul(
                        psv[:, 64 * s2:64 * s2 + 64],
                        v_sb[:, 1024 * c + s + s2:1024 * c + s + s2 + 1009:16],
                        qw_sb[:, WOFF + 160 * c + 64:WOFF + 160 * c + 128],
                        start=(c == 0), stop=(c == 1),
                    )
            pv = psv[:].rearrange("r (s2 d c2) -> r s2 d c2", s2=2, c2=16)
            dst = Vtm[:].rearrange("p (h t c) -> p h t c", t=8, c=8)
            nc.vector.tensor_copy(dst[0:64, s:s + 2, :, 4:8],
                                  pv[:, :, :, 0:16:2].transpose([0, 1, 3, 2]))
            nc.vector.tensor_copy(dst[64:128, s:s + 2, :, 4:8],
                                  pv[:, :, :, 1:16:2].transpose([0, 1, 3, 2]))

        # ---------------- phase 2: attention ----------------
        # scores: per m-chunk t one M=128 K=4 matmul at tile row 32*(t%4);
        # four consecutive t land on four distinct PE quadrants -> concurrent
        Kv = [Kp2[32 * rg:32 * rg + 4, :].rearrange("d (m s) -> d m s", s=16)
              for rg in range(4)]
        Qv = [Qp2[32 * rg:32 * rg + 4, :] for rg in range(4)]
        for h in range(H):
            bh2 = expb_tiles[h // 2]
            hb = 4096 * (h % 2)
            es = ep.tile([128, 4096], BF16, tag="es")
            for u in range(4):   # pairs of m-chunks -> one 2-bank psum tile
                ps = pss.tile([128, 1024], F32, tag="ps")
                for v2 in range(2):
                    t = 2 * u + v2
                    rg = t % 4
                    nc.tensor.matmul(ps[:, 512 * v2:512 * v2 + 512],
                                     Kv[rg][:, 128 * t:128 * t + 128, h],
                                     Qv[rg][:, 512 * h:512 * h + 512],
                                     start=True, stop=True,
                                     tile_position=(32 * rg, 0))
                nc.scalar.activation(es[:, 1024 * u:1024 * u + 1024], ps[:], AF.Exp)
            at = ap.tile([128, 4096], BF16, tag="at")
            nc.vector.tensor_mul(at[:, 0:3072], es[:, 0:3072], bh2[:, hb:hb + 3072])
            nc.gpsimd.tensor_mul(at[:, 3072:4096], es[:, 3072:4096],
                                 bh2[:, hb + 3072:hb + 4096])
            # attn@V, flipped: the attn [128m, 128n] chunk is the stationary
            # operand (contiguous bf16 128-col weight loads -> FWL) and only
            # the 8 Vtm columns stream. Output lands n-on-partitions: psum
            # [128n, (denominator x4 | x x4)] per n-block, two n-blocks per
            # 2-bank psum tile (cols 0.. and 512.. -> separate banks, so the
            # per-block start=True bank-clears don't interact).
            # Both n-blocks of a half share ONE psum bank (cols 0..7 / 8..15):
            # the first matmul's start=True clears the whole bank's
            # has_written bits, so the second block's t=0 (start=False)
            # overwrites rather than accumulating stale data -- PE executes
            # in order, making this deterministic.
            pns = []
            for half in range(2):
                pn = psx.tile([128, 512], F32, tag="pn")
                for t in range(8):
                    for nb2 in range(2):
                        nb = 2 * half + nb2
                        nc.tensor.matmul(
                            pn[:, 8 * nb2:8 * nb2 + 8],
                            at[:, 512 * t + 128 * nb:512 * t + 128 * nb + 128],
                            Vtm[:, 64 * h + 8 * t:64 * h + 8 * t + 8],
                            start=(t == 0 and nb2 == 0), stop=(t == 7),
                            skip_group_check=True)
                pns.append(pn)
            # normalize: cols 0..3 hold the denominator replicated 4x, so one
            # reciprocal + one multiply per psum tile cover 2 n-blocks
            xv = xTs[:].rearrange("p (nb c) -> p nb c", nb=4)
            for half, pn in enumerate(pns):
                pv8 = pn[:, 0:16].rearrange("p (nb c) -> p nb c", nb=2)
                rec = sp.tile([128, 8], F32, tag="rec")
                rv = rec[:].rearrange("p (nb c) -> p nb c", nb=2)
                nc.vector.reciprocal_approx_fast(rv, pv8[:, :, 0:4])
                nc.vector.tensor_mul(
                    xv[:, 2 * half:2 * half + 2, 4 * h:4 * h + 4],
                    pv8[:, :, 4:8], rv)
            # fetch the next exp-bias pair once the gpsimd mult above has
            # freed its pool slot (keeps the WAR wait behind the consumers
            # in the gpsimd instruction stream)
            if h % 2 == 1 and 4 + (h - 1) // 2 < 8:
                fetch_pair(4 + (h - 1) // 2)

        # ---------------- phase 3: transpose x, output conv + BN + LeakyReLU ----
        # xTs [128n, (nb, c)] -> x_sb [64c, 512n] via 4 PE transposes
        for nb in range(4):
            pt = pss.tile([64, 128], FP16, tag="ps")
            nc.tensor.transpose(pt[:], xTs[:, 64 * nb:64 * nb + 64], ident[:])
            nc.vector.tensor_copy(x_sb[:, 128 * nb:128 * nb + 128], pt[:])
        for u in range(2):
            psy = pss.tile([128, NH], F32, tag="ps")
            nc.tensor.matmul(psy[:], woT_sb[0:64, 128 * u:128 * u + 128], x_sb[:],
                             start=True, stop=True)
            y2 = sp.tile([128, NH], F32, tag="y2")
            nc.vector.tensor_scalar(y2[:], psy[:], bn_sb[:, u:u + 1], bn_sb[:, 2 + u:3 + u],
                                    ALU.mult, ALU.add)
            yt = sp.tile([128, NH], F32, tag="yt")
            nc.vector.scalar_tensor_tensor(yt[:], y2[:], NEG_SLOPE, y2[:],
                                           ALU.mult, ALU.max)
            nc.sync.dma_start(y[u], yt[:])


def build_program():
    nc = bacc.Bacc("TRN2", target_bir_lowering=False, debug=False)
    io = {
        "qkvw": nc.dram_tensor("qkvw", [128, QKVW_W], FP16,
                               kind="ExternalInput").ap(),
        "expbT": nc.dram_tensor("expbT", [8, 128, 2, 8, NH], BF16,
                                kind="ExternalInput").ap(),
        "woT": nc.dram_tensor("woT", [64, HID], F32, kind="ExternalInput").ap(),
        "bnst": nc.dram_tensor("bnst", [128, 4], F32, kind="ExternalInput").ap(),
        "ident": nc.dram_tensor("ident", [128, 128], FP16, kind="ExternalInput").ap(),
        "y": nc.dram_tensor("y", [2, 128, NH], F32, kind="ExternalOutput").ap(),
    }
    with tile.TileContext(nc) as tc:
        _emit(nc, tc, io)
    nc.compile()
    return nc


def make_in_maps(q, k, v, attn_bias, Wq, Wk, Wv, Wo, bo, gamma, beta, run_mean, run_var):
    def f32(x):
        return np.ascontiguousarray(np.asarray(x, dtype=np.float32))

    bf16 = ml_dtypes.bfloat16
    q, k, v = f32(q), f32(k), f32(v)
    Wq, Wk, Wv, Wo, bo = f32(Wq), f32(Wk), f32(Wv), f32(Wo), f32(bo)
    gamma, beta, run_mean, run_var = f32(gamma), f32(beta), f32(run_mean), f32(run_var)

    woT = f32(Wo.T)
    # BN affine precomputed on host: s = gamma*rsqrt(var+eps),
    # t = (bo - mean)*s + beta; packed [128, (s0,s1,t0,t1)]
    s = gamma / np.sqrt(run_var + BN_EPS)
    t = (bo - run_mean) * s + beta
    bnst = f32(np.concatenate([s.reshape(2, 128).T, t.reshape(2, 128).T], axis=1))

    wkT16 = Wk.T.astype(np.float16)     # [256, 64]
    wvT16 = Wv.T.astype(np.float16)

    # exp of the additive bias, in bf16 (softmax(s+b) = exp(s)*exp(b))
    expb = np.exp(np.asarray(attn_bias, dtype=np.float32)).astype(bf16)

    def fold(a, width):
        return a.reshape(2, 128, width).transpose(1, 0, 2).reshape(128, 2 * width)

    in_maps = []
    for core in range(NCORES):
        b, half = divmod(core, 2)
        n0 = half * NH
        rows = np.array([16 * d + 8 * half + jl for jl in range(8) for d in range(4)])
        wqT16 = Wq[rows, :].T.astype(np.float16)                  # [256, 32]
        # packed [128p, (k | wk,wv,wq | q | v)], fp16, c2-folded per partition
        qkvw = np.empty((128, QKVW_W), dtype=np.float16)
        qkvw[:, KOFF:KOFF + 2 * N] = fold(k[b], N)
        wcat = np.concatenate([wkT16, wvT16, wqT16], axis=1)      # [256, 160]
        qkvw[:, WOFF:WOFF + 320] = fold(wcat, 160)
        qkvw[:, QOFF:QOFF + 2 * N] = fold(q[b], N)
        qkvw[:, VOFF:VOFF + 2 * N] = fold(v[b], N)

        # expbT [pair, p, h2, t, n]: exp(bias)[m = 128t + p, n-half]
        bt = expb[b, :, n0:n0 + NH, :].transpose(0, 2, 1)         # [16h, 1024m, 512n]
        expbT = np.ascontiguousarray(
            bt.reshape(8, 2, 8, 128, NH).transpose(0, 3, 1, 2, 4))
        in_maps.append({
            "qkvw": np.ascontiguousarray(qkvw),
            "expbT": expbT,
            "woT": woT, "bnst": bnst,
            "ident": np.eye(128, dtype=np.float16),
        })
    return in_maps


_NC_CACHE = None


def get_nc():
    global _NC_CACHE
    if _NC_CACHE is None:
        _NC_CACHE = build_program()
    return _NC_CACHE


def kernel(**inputs):
    nc = get_nc()
    in_maps = make_in_maps(**inputs)
    res = run_bass_kernel_spmd(nc, in_maps, list(range(NCORES)))
    out = np.empty((B, HID, N), dtype=np.float32)
    for core in range(NCORES):
        b, half = divmod(core, 2)
        out[b, :, half * NH:(half + 1) * NH] = \
            res.results[core]["y"].reshape(HID, NH)
    return out


# revision 42
# speedup vs baseline: 1.1813x; 1.0435x over previous
"""Trainium2 Bass kernel for nn_MultiHeadAttention_80418967650946.

Reference computation (per batch b):
  qp/kp/vp = 1x1-conv projections of q/k/v   [64, N]
  funky head view: qh[h,n,d] = qp.reshape(4, 16*N)[d, 16n+h]  (same for kh, vh)
  scores = qh @ kh * 0.25^0.5 + bias ; attn = softmax(scores)
  x[4h+d, n] = (attn @ vh)[h, n, d] ; y = LeakyReLU(BN(Wo @ x + bo), 0.2)

Sharding: 8 cores = 4 batches x 2 query-halves (n in [0,512) or [512,1024)).
Each core computes its query-half for ALL 16 heads fully locally (no
collectives): the output conv is column-wise independent, so y[:, n-half]
only needs x[:, n-half].

Key structure (engine-balance driven; ACT exp of 8.4M scores ~72us is the
compute wall, bias HBM stream ~16 MiB is the memory wall):
  - softmax(s + b) = exp(s) * exp(b) with exp(b) precomputed on the HOST in
    bf16: halves bias HBM traffic and turns the bias add into 16-bit
    multiplies, split DVE (half A) / GpSimd (half B) -- tensor_tensor never
    contends with the shared DVE/GpSimd SBUF port pair.
  - scores psum from K=4 matmuls packed 4-concurrent via tile_position row
    groups (Kp2/Qp2 replicated at partitions 32*rg).
  - K-projection evacuates psum via full-partition [128,1024] copies into a
    staging tile; the 4-partition consolidation + row-group replication is
    16 HWDGE sbuf->sbuf DMAs (4-lane DVE copies and whole-row replication
    DMAs are both ~27 GB/s engine-pinned; this splits and overlaps them).
  - attn@V lhsT has ones in cols 0..3 and V in cols 32..35 (M=36): softmax
    denominator lands on psum partitions 0..3 and x on 32..35, both legal
    32-aligned engine AP bases -> normalization is pure DVE.
  - BN affine precomputed on host; input qkv+weights packed fp16, one
    contiguous run per partition per DMA.
"""
import sys

if "/opt/trn_rl_repo" not in sys.path:
    sys.path.insert(0, "/opt/trn_rl_repo")

import numpy as np
import ml_dtypes

import concourse.bass as bass
import concourse.tile as tile
from concourse import bacc, mybir
from concourse.bass_utils import run_bass_kernel_spmd

F32 = mybir.dt.float32
F32R = mybir.dt.float32r
BF16 = mybir.dt.bfloat16
FP16 = mybir.dt.float16
AF = mybir.ActivationFunctionType
ALU = mybir.AluOpType
PSUM = bass.MemorySpace.PSUM

H = 16
D = 4
HID = 256
B = 4
N = 1024
NH = 512          # per-core query positions
NCORES = 8
SCALE = float(D) ** -0.5
BN_EPS = 1e-5
NEG_SLOPE = 0.2

# packed input column offsets (fp16 elements): [k | w | q | v]
KOFF = 0
WOFF = 2 * N
QOFF = 2 * N + 320
VOFF = 4 * N + 320
QKVW_W = 6 * N + 320


def _emit(nc, tc, io):
    qkvw, expbT = io["qkvw"], io["expbT"]
    woT, bnst, y = io["woT"], io["bnst"], io["y"]

    with (
        tc.tile_pool(name="persist", bufs=1) as persist,
        tc.tile_pool(name="expb", bufs=4) as bp,
        tc.tile_pool(name="exps", bufs=2) as ep,
        tc.tile_pool(name="attn", bufs=2) as ap,
        tc.tile_pool(name="sml", bufs=4) as sp,
        tc.tile_pool(name="p1", bufs=1) as p1,
        tc.tile_pool(name="ps_s", bufs=3, space=PSUM) as pss,
        tc.tile_pool(name="ps_x", bufs=2, space=PSUM) as psx,
    ):
        Kst = persist.tile([128, 4096], BF16, tag="Kst")
        Kp2 = persist.tile([100, H * N], BF16, tag="Kp2")
        Qp2 = persist.tile([100, H * NH], BF16, tag="Qp2")
        Vtm = persist.tile([128, H * 8 * 8], BF16, tag="Vtm")
        xTs = persist.tile([128, 256], FP16, tag="xTs")
        x_sb = persist.tile([64, NH], F32R, tag="x_sb")
        woT_sb = persist.tile([64, HID], F32R, tag="woT_sb")
        bn_sb = persist.tile([128, 4], F32, tag="bn_sb")
        ident = persist.tile([128, 128], FP16, tag="ident")

        # ---------------- phase 0: input DMAs ----------------
        # k+weights first (K projection starts earliest), then q, then v;
        # all on the scalar HWDGE queue, one contiguous run per partition.
        qw_sb = p1.tile([128, QKVW_W], FP16, tag="qw_sb")
        nc.sync.dma_start(qw_sb[:, KOFF:QOFF], qkvw[:, KOFF:QOFF])
        nc.scalar.dma_start(qw_sb[:, QOFF:VOFF], qkvw[:, QOFF:VOFF])
        nc.scalar.dma_start(qw_sb[:, VOFF:QKVW_W], qkvw[:, VOFF:QKVW_W])
        k_sb = qw_sb[:, KOFF:KOFF + 2 * N]
        q_sb = qw_sb[:, QOFF:QOFF + 2 * N]
        v_sb = qw_sb[:, VOFF:VOFF + 2 * N]

        nc.gpsimd.dma_start(woT_sb[:], woT)
        nc.scalar.dma_start(bn_sb[:], bnst)
        nc.scalar.dma_start(ident[:], io["ident"])

        # dummy exp: pulls the ~2.7us ACT_TABLE_LOAD for the Exp set off the
        # first real exp's critical path (it loads while input DMAs stream)
        warm = p1.tile([1, 8], F32, tag="warm")
        nc.vector.memset(warm[:], 0.0)
        nc.scalar.activation(warm[:], warm[:], AF.Exp)

        # exp-bias prefetch on the sync HWDGE queue (no cast involved, and it
        # keeps DMA-issue work off the gpsimd stream, which now only runs
        # multiplies). Pair 0 queues first; pairs 1-3 queue after the
        # consolidation DMAs (sync HWDGE is FIFO per queue, so a 2 MiB pair
        # must not sit in front of head-0's Kp2/Qp2 pieces); pairs 4-7 are
        # emitted inside the head loop once their pool slot is free.
        expb_tiles = []

        def fetch_pair(pair):
            bh2 = bp.tile([128, 2 * 8 * NH], BF16, tag="bh2")
            nc.sync.dma_start(
                bh2[:].rearrange("p (h t n) -> p h t n", h=2, t=8),
                expbT[pair])
            expb_tiles.append(bh2)

        fetch_pair(0)

        # ---------------- phase 1: projections ----------------
        # Q proj first (its epilogue + replication chain is longest): 4
        # j-values col-tiled per [128,1024] psum tile; epilogue copies
        # reorder to head-major Qp2 (scaled by SCALE); after each b4 half, 3
        # strided row-group replication DMAs for that half (Qp2 free =
        # 512*head + 256*b4 + 64*g + a).
        for b4 in range(2):
            psq = pss.tile([128, 1024], F32, tag="ps")
            for g in range(4):
                j = 4 * b4 + g
                for nn2 in range(2):
                    for c in range(2):
                        nc.tensor.matmul(
                            psq[32 * g:32 * g + 4, 512 * nn2:512 * nn2 + 512],
                            qw_sb[:, WOFF + 160 * c + 128 + 4 * j:
                                  WOFF + 160 * c + 128 + 4 * j + 4],
                            q_sb[:, 1024 * c + 512 * nn2:1024 * c + 512 * nn2 + 512],
                            start=(c == 0), stop=(c == 1), tile_position=(0, 32 * g))
            for g in range(4):
                j = 4 * b4 + g
                srcv = psq[32 * g:32 * g + 4, :].rearrange("d (a b) -> d b a", b=16)
                dstv = Qp2[0:4, :].rearrange("d (b q) -> d b q", b=16)[:, :, 64 * j:64 * j + 64]
                if g % 2 == 0:
                    nc.vector.tensor_scalar_mul(dstv, srcv, SCALE)
                else:
                    nc.scalar.mul(dstv, srcv, SCALE)
            for rep in range(1, 4):
                nc.sync.dma_start(
                    Qp2[32 * rep:32 * rep + 4, :].rearrange(
                        "d (b hf x) -> d b hf x", b=16, hf=2)[:, :, b4, :],
                    Qp2[0:4, :].rearrange(
                        "d (b hf x) -> d b hf x", b=16, hf=2)[:, :, b4, :])

        # K proj: same col-tiling. Evacuate each b4 with ONE full-partition
        # copy into Kst[:, 1024*b4]; consolidation into the Kp2 row groups is
        # 8 HWDGE DMAs (row group rr only ever reads g = j%4 in {0,1} for
        # even rr / {2,3} for odd rr: Kp2[32rr+d, 1024*(4b4+g)+n] =
        # Kst[32g+d, 1024b4+n]).
        KpV = Kp2[:].rearrange("p (b4 g c) -> p b4 g c", b4=4, g=4)
        KsV = Kst[:].rearrange("p (b4 c) -> p b4 c", b4=4)
        for b4 in range(4):
            psk = pss.tile([128, 1024], F32, tag="ps")
            for g in range(4):
                j = 4 * b4 + g
                for nn2 in range(2):
                    for c in range(2):
                        nc.tensor.matmul(
                            psk[32 * g:32 * g + 4, 512 * nn2:512 * nn2 + 512],
                            qw_sb[:, WOFF + 160 * c + j:WOFF + 160 * c + j + 49:16],
                            k_sb[:, 1024 * c + 512 * nn2:1024 * c + 512 * nn2 + 512],
                            start=(c == 0), stop=(c == 1), tile_position=(0, 32 * g))
            if b4 % 2 == 0:
                nc.vector.tensor_copy(Kst[:, 1024 * b4:1024 * b4 + 1024], psk[:])
            else:
                nc.scalar.copy(Kst[:, 1024 * b4:1024 * b4 + 1024], psk[:])
        for rr in range(4):
            for g in (0, 1) if rr % 2 == 0 else (2, 3):
                nc.sync.dma_start(KpV[32 * rr:32 * rr + 4, :, g, :],
                                  KsV[32 * g:32 * g + 4, :, :])
        for pair in range(1, 4):
            fetch_pair(pair)

        # V projection into Vtm [128, (h, t, c8)] bf16: c 0..3 = 1.0 (the 4
        # ones columns make the softmax denominator land 4-fold replicated in
        # psum free cols 0..3), c = 4+d holds vh[m = 128t + p, d] for head h.
        # Two heads per psum tile halve the (strided, overhead-bound)
        # evacuation copy count.
        nc.vector.memset(
            Vtm[:].rearrange("p (h t c) -> p h t c", t=8, c=8)[:, :, :, 0:4], 1.0)
        for s in range(0, 16, 2):
            psv = psx.tile([64, 128], F32, tag="pn")
            for s2 in range(2):
                for c in range(2):
                    nc.tensor.matmul(
                        psv[:, 64 * s2:64 * s2 + 64],
                        v_sb[:, 1024 * c + s + s2:1024 * c + s + s2 + 1009:16],
                        qw_sb[:, WOFF + 160 * c + 64:WOFF + 160 * c + 128],
                        start=(c == 0), stop=(c == 1),
                    )
            pv = psv[:].rearrange("r (s2 d c2) -> r s2 d c2", s2=2, c2=16)
            dst = Vtm[:].rearrange("p (h t c) -> p h t c", t=8, c=8)
            nc.vector.tensor_copy(dst[0:64, s:s + 2, :, 4:8],
                                  pv[:, :, :, 0:16:2].transpose([0, 1, 3, 2]))
            nc.vector.tensor_copy(dst[64:128, s:s + 2, :, 4:8],
                                  pv[:, :, :, 1:16:2].transpose([0, 1, 3, 2]))

        # ---------------- phase 2: attention ----------------
        # scores: per m-chunk t one M=128 K=4 matmul at tile row 32*(t%4);
        # four consecutive t land on four distinct PE quadrants -> concurrent
        Kv = [Kp2[32 * rg:32 * rg + 4, :].rearrange("d (m s) -> d m s", s=16)
              for rg in range(4)]
        Qv = [Qp2[32 * rg:32 * rg + 4, :] for rg in range(4)]
        for h in range(H):
            bh2 = expb_tiles[h // 2]
            hb = 4096 * (h % 2)
            es = ep.tile([128, 4096], BF16, tag="es")
            for u in range(4):   # pairs of m-chunks -> one 2-bank psum tile
                ps = pss.tile([128, 1024], F32, tag="ps")
                for v2 in range(2):
                    t = 2 * u + v2
                    rg = t % 4
                    nc.tensor.matmul(ps[:, 512 * v2:512 * v2 + 512],
                                     Kv[rg][:, 128 * t:128 * t + 128, h],
                                     Qv[rg][:, 512 * h:512 * h + 512],
                                     start=True, stop=True,
                                     tile_position=(32 * rg, 0))
                nc.scalar.activation(es[:, 1024 * u:1024 * u + 1024], ps[:], AF.Exp)
            at = ap.tile([128, 4096], BF16, tag="at")
            nc.vector.tensor_mul(at[:, 0:3072], es[:, 0:3072], bh2[:, hb:hb + 3072])
            nc.gpsimd.tensor_mul(at[:, 3072:4096], es[:, 3072:4096],
                                 bh2[:, hb + 3072:hb + 4096])
            # attn@V, flipped: the attn [128m, 128n] chunk is the stationary
            # operand (contiguous bf16 128-col weight loads -> FWL) and only
            # the 8 Vtm columns stream. Output lands n-on-partitions: psum
            # [128n, (denominator x4 | x x4)] per n-block, two n-blocks per
            # 2-bank psum tile (cols 0.. and 512.. -> separate banks, so the
            # per-block start=True bank-clears don't interact).
            # Both n-blocks of a half share ONE psum bank (cols 0..7 / 8..15):
            # the first matmul's start=True clears the whole bank's
            # has_written bits, so the second block's t=0 (start=False)
            # overwrites rather than accumulating stale data -- PE executes
            # in order, making this deterministic.
            pns = []
            for half in range(2):
                pn = psx.tile([128, 512], F32, tag="pn")
                for t in range(8):
                    for nb2 in range(2):
                        nb = 2 * half + nb2
                        nc.tensor.matmul(
                            pn[:, 8 * nb2:8 * nb2 + 8],
                            at[:, 512 * t + 128 * nb:512 * t + 128 * nb + 128],
                            Vtm[:, 64 * h + 8 * t:64 * h + 8 * t + 8],
                            start=(t == 0 and nb2 == 0), stop=(t == 7),
                            skip_group_check=True)
                pns.append(pn)
            # normalize: cols 0..3 hold the denominator replicated 4x, so one
            # reciprocal + one multiply per psum tile cover 2 n-blocks
            xv = xTs[:].rearrange("p (nb c) -> p nb c", nb=4)
            for half, pn in enumerate(pns):
                pv8 = pn[:, 0:16].rearrange("p (nb c) -> p nb c", nb=2)
                rec = sp.tile([128, 8], F32, tag="rec")
                rv = rec[:].rearrange("p (nb c) -> p nb c", nb=2)
                nc.vector.reciprocal_approx_fast(rv, pv8[:, :, 0:4])
                nc.vector.tensor_mul(
                    xv[:, 2 * half:2 * half + 2, 4 * h:4 * h + 4],
                    pv8[:, :, 4:8], rv)
            # fetch the next exp-bias pair once the gpsimd mult above has
            # freed its pool slot (keeps the WAR wait behind the consumers
            # in the gpsimd instruction stream)
            if h % 2 == 1 and 4 + (h - 1) // 2 < 8:
                fetch_pair(4 + (h - 1) // 2)

        # ---------------- phase 3: transpose x, output conv + BN + LeakyReLU ----
        # xTs [128n, (nb, c)] -> x_sb [64c, 512n] via 4 PE transposes
        for nb in range(4):
            pt = pss.tile([64, 128], FP16, tag="ps")
            nc.tensor.transpose(pt[:], xTs[:, 64 * nb:64 * nb + 64], ident[:])
            nc.vector.tensor_copy(x_sb[:, 128 * nb:128 * nb + 128], pt[:])
        for u in range(2):
            psy = pss.tile([128, NH], F32, tag="ps")
            nc.tensor.matmul(psy[:], woT_sb[0:64, 128 * u:128 * u + 128], x_sb[:],
                             start=True, stop=True)
            y2 = sp.tile([128, NH], F32, tag="y2")
            nc.vector.tensor_scalar(y2[:], psy[:], bn_sb[:, u:u + 1], bn_sb[:, 2 + u:3 + u],
                                    ALU.mult, ALU.add)
            yt = sp.tile([128, NH], F32, tag="yt")
            nc.vector.scalar_tensor_tensor(yt[:], y2[:], NEG_SLOPE, y2[:],
                                           ALU.mult, ALU.max)
            nc.sync.dma_start(y[u], yt[:])


def build_program():
    nc = bacc.Bacc("TRN2", target_bir_lowering=False, debug=False)
    io = {
        "qkvw": nc.dram_tensor("qkvw", [128, QKVW_W], FP16,
                               kind="ExternalInput").ap(),
        "expbT": nc.dram_tensor("expbT", [8, 128, 2, 8, NH], BF16,
                                kind="ExternalInput").ap(),
        "woT": nc.dram_tensor("woT", [64, HID], F32, kind="ExternalInput").ap(),
        "bnst": nc.dram_tensor("bnst", [128, 4], F32, kind="ExternalInput").ap(),
        "ident": nc.dram_tensor("ident", [128, 128], FP16, kind="ExternalInput").ap(),
        "y": nc.dram_tensor("y", [2, 128, NH], F32, kind="ExternalOutput").ap(),
    }
    with tile.TileContext(nc) as tc:
        _emit(nc, tc, io)
    nc.compile()
    return nc


def make_in_maps(q, k, v, attn_bias, Wq, Wk, Wv, Wo, bo, gamma, beta, run_mean, run_var):
    def f32(x):
        return np.ascontiguousarray(np.asarray(x, dtype=np.float32))

    bf16 = ml_dtypes.bfloat16
    q, k, v = f32(q), f32(k), f32(v)
    Wq, Wk, Wv, Wo, bo = f32(Wq), f32(Wk), f32(Wv), f32(Wo), f32(bo)
    gamma, beta, run_mean, run_var = f32(gamma), f32(beta), f32(run_mean), f32(run_var)

    woT = f32(Wo.T)
    # BN affine precomputed on host: s = gamma*rsqrt(var+eps),
    # t = (bo - mean)*s + beta; packed [128, (s0,s1,t0,t1)]
    s = gamma / np.sqrt(run_var + BN_EPS)
    t = (bo - run_mean) * s + beta
    bnst = f32(np.concatenate([s.reshape(2, 128).T, t.reshape(2, 128).T], axis=1))

    wkT16 = Wk.T.astype(np.float16)     # [256, 64]
    wvT16 = Wv.T.astype(np.float16)

    # exp of the additive bias, in bf16 (softmax(s+b) = exp(s)*exp(b))
    expb = np.exp(np.asarray(attn_bias, dtype=np.float32)).astype(bf16)

    def fold(a, width):
        return a.reshape(2, 128, width).transpose(1, 0, 2).reshape(128, 2 * width)

    in_maps = []
    for core in range(NCORES):
        b, half = divmod(core, 2)
        n0 = half * NH
        rows = np.array([16 * d + 8 * half + jl for jl in range(8) for d in range(4)])
        wqT16 = Wq[rows, :].T.astype(np.float16)                  # [256, 32]
        # packed [128p, (k | wk,wv,wq | q | v)], fp16, c2-folded per partition
        qkvw = np.empty((128, QKVW_W), dtype=np.float16)
        qkvw[:, KOFF:KOFF + 2 * N] = fold(k[b], N)
        wcat = np.concatenate([wkT16, wvT16, wqT16], axis=1)      # [256, 160]
        qkvw[:, WOFF:WOFF + 320] = fold(wcat, 160)
        qkvw[:, QOFF:QOFF + 2 * N] = fold(q[b], N)
        qkvw[:, VOFF:VOFF + 2 * N] = fold(v[b], N)

        # expbT [pair, p, h2, t, n]: exp(bias)[m = 128t + p, n-half]
        bt = expb[b, :, n0:n0 + NH, :].transpose(0, 2, 1)         # [16h, 1024m, 512n]
        expbT = np.ascontiguousarray(
            bt.reshape(8, 2, 8, 128, NH).transpose(0, 3, 1, 2, 4))
        in_maps.append({
            "qkvw": np.ascontiguousarray(qkvw),
            "expbT": expbT,
            "woT": woT, "bnst": bnst,
            "ident": np.eye(128, dtype=np.float16),
        })
    return in_maps


_NC_CACHE = None


def get_nc():
    global _NC_CACHE
    if _NC_CACHE is None:
        _NC_CACHE = build_program()
    return _NC_CACHE


def kernel(**inputs):
    nc = get_nc()
    in_maps = make_in_maps(**inputs)
    res = run_bass_kernel_spmd(nc, in_maps, list(range(NCORES)))
    out = np.empty((B, HID, N), dtype=np.float32)
    for core in range(NCORES):
        b, half = divmod(core, 2)
        out[b, :, half * NH:(half + 1) * NH] = \
            res.results[core]["y"].reshape(HID, NH)
    return out
